# revision 13
# baseline (speedup 1.0000x reference)
"""Trainium2 Bass kernel for nn_DecoderLayer (Transformer-XL style decoder layer).

Sharding (8 cores = 2 batch groups x 4-way tensor parallel), core c:
b = c // 4, g = c % 4 (4 local heads, 256 q/k/v features per core).

Structure (v2 — chunk-pipelined, fp8 DoubleRow):
  - Projections: fp8 DoubleRow matmuls (K=256 per instr) from host-prepped
    fp8 ct/wq/wk/wv. Q^T and K^T evicted to bf16; V is computed by a second
    natural-layout projection straight into fp8 [j, head, d|denom] tiles.
  - Attention runs over 4 t-chunks of 512 rows; per (chunk, head) the
    S (bf16) -> exp (ACT, fused *1/8 scale + ln16 bias, fp8 out) -> P@V
    (fp8 DoubleRow over j-block pairs) pipeline accumulates attn_vec plus
    the softmax denominator (1/64-scaled ones column) in one PSUM group.
    Normalize = DVE reciprocal + gpsimd partition_broadcast + DVE mul.
  - Wo: fp8 DoubleRow (Wo*16 on host, avT*64 from normalize, /1024 on evict),
    partials cast to bf16, chunked ReduceScatter per 512 rows.
  - FF (bf16, full precision path): per-chunk LN1 + transposes; FF1/FF2
    matmul quanta are interleaved into later chunks' attention at head
    boundaries to fill PE gaps left by the exp-bound softmax; FF2
    accumulates in SBUF via scalar_tensor_tensor so PSUM stays within
    8 banks alongside attention.
"""

import math
import sys

sys.path.insert(0, "/opt/trn_rl_repo")

from contextlib import ExitStack

import numpy as np
import ml_dtypes

import concourse.bass as bass
import concourse.bacc as bacc
import concourse.mybir as mybir
import concourse.tile as tile
from concourse.bass_utils import run_bass_kernel_spmd
from concourse.masks import make_identity

T, M, B, D, H, DH, DI = 2048, 1024, 2, 1024, 16, 64, 4096
TM = T + M
NCORES = 8
G = 4                # tensor-parallel group size
HL = H // G          # 4 local heads
HDH_L = HL * DH      # 256 local q/k/v features
TQ = T // G          # 512 t-rows per core after ReduceScatter
NB_J = TM // 128     # 24 key blocks
NC_T = 4             # t-chunks of 512
NEG = -1.0e9
SCALE = 1.0 / float(DH) ** 0.5
LNBIAS = math.log(16.0) - 1.0  # fp8 headroom: max logit+bias ~4.8 < ln(240)

BF16 = mybir.dt.bfloat16
F32 = mybir.dt.float32
FP8 = mybir.dt.float8e4
NPBF16 = ml_dtypes.bfloat16
NPFP8 = ml_dtypes.float8_e4m3
DR = mybir.MatmulPerfMode.DoubleRow

_prog_cache = {}


def _bf(x):
    return np.ascontiguousarray(np.asarray(x, dtype=np.float32).astype(NPBF16))


def _f8(x):
    return np.ascontiguousarray(np.asarray(x, dtype=np.float32).astype(NPFP8))


def _f32(x):
    return np.ascontiguousarray(np.asarray(x, dtype=np.float32))


def build_program(fvt, mask_list, trace=False):
    """fvt[bj] = first visible t-block (0..16; 16 = column fully masked).
    mask_list = tuple of (bj, tb) pairs needing an additive mask tile."""
    fvt = list(fvt)
    n_mask = max(len(mask_list), 1)
    AF = mybir.ActivationFunctionType
    ALU = mybir.AluOpType

    nc = bacc.Bacc(None, target_bir_lowering=False, num_devices=NCORES)

    ct8_d = nc.dram_tensor("ct8", [4, 128, 2, TM], FP8, kind="ExternalInput")
    wq8_d = nc.dram_tensor("wq8", [4, 128, 2, HDH_L], FP8, kind="ExternalInput")
    wk8_d = nc.dram_tensor("wk8", [4, 128, 2, HDH_L], FP8, kind="ExternalInput")
    wv8_d = nc.dram_tensor("wv8", [4, 128, 2, HDH_L], FP8, kind="ExternalInput")
    wo8_d = nc.dram_tensor("wo8", [128, 2, D], FP8, kind="ExternalInput")
    w1_d = nc.dram_tensor("w1", [32, 128, 8, 128], BF16, kind="ExternalInput")
    w2_d = nc.dram_tensor("w2", [32, 128, D], BF16, kind="ExternalInput")
    mask_d = nc.dram_tensor("maskt", [n_mask, 128, 128], BF16, kind="ExternalInput")
    hres_d = nc.dram_tensor("hres", [4, 128, D], BF16, kind="ExternalInput")
    lnp_d = nc.dram_tensor("lnp", [4, 128, D], BF16, kind="ExternalInput")
    b1t_d = nc.dram_tensor("b1t", [128, 32], F32, kind="ExternalInput")
    b2b_d = nc.dram_tensor("b2b", [128, D], BF16, kind="ExternalInput")
    out_d = nc.dram_tensor("out", [4, 128, D], F32, kind="ExternalOutput")

    # visible j-blocks per chunk
    def jbs_of(c):
        return [jb for jb in range(NB_J)
                if fvt[jb] < 16 and fvt[jb] * 128 < (c + 1) * 512]

    mask_by = {}   # (c) -> list of (jb, tb, mask_idx)
    for i, (bj, tb) in enumerate(mask_list):
        c = tb // 4
        mask_by.setdefault(c, []).append((bj, tb, i))

    with ExitStack() as ctx:
        tc = ctx.enter_context(tile.TileContext(nc))
        per = ctx.enter_context(tc.tile_pool(name="per", bufs=1))
        dram = ctx.enter_context(tc.tile_pool(name="dram", bufs=1, space="DRAM"))

        # ---------- persistent SBUF ----------
        b1t_s = per.tile([128, 32], F32, tag="b1t", name="b1t")
        b2b_s = per.tile([128, D], BF16, tag="b2b", name="b2b")
        ln_s = [per.tile([128, D], BF16, tag=f"ln{i}", name=f"ln{i}") for i in range(4)]
        hres_s = [per.tile([128, D], BF16, tag=f"hres{i}", name=f"hres{i}")
                  for i in range(4)]
        eps_s = per.tile([128, 1], F32, tag="eps", name="eps")
        lnb_s = per.tile([128, 1], F32, tag="lnb", name="lnb")
        ident = per.tile([128, 128], BF16, tag="ident", name="ident")

        attn_cm = tc.tile_pool(name="attn", bufs=1)
        attn = attn_cm.__enter__()
        qT = [attn.tile([128, T], BF16, tag=f"qT{m}", name=f"qT{m}") for m in range(2)]
        kT = [attn.tile([128, TM], BF16, tag=f"kT{m}", name=f"kT{m}") for m in range(2)]
        # per-head V block padded 65->68 so the DoubleRow k-tile stride
        # (4*68=272 bytes) meets the perf-mode 16-byte alignment rule
        v8 = attn.tile([128, NB_J // 2, 2, HL, DH + 4], FP8, tag="v8", name="v8")
        mk_s = [attn.tile([128, 128], BF16, tag=f"mk{i}", name=f"mk{i}")
                for i in range(len(mask_list))]
        wo8_s = attn.tile([128, 2, D], FP8, tag="wo8", name="wo8")

        # FF-side persistent
        xT = [per.tile([128, TQ], BF16, tag=f"xT{k}", name=f"xT{k}") for k in range(8)]
        x_s = [per.tile([128, D], BF16, tag=f"x{k}", name=f"x{k}") for k in range(4)]
        rT = [per.tile([128, TQ], BF16, tag=f"rT{k}", name=f"rT{k}") for k in range(32)]
        f2sb = [per.tile([128, D], BF16, tag=f"f2sb{k}", name=f"f2sb{k}")
                for k in range(4)]

        rs_in = dram.tile([16, 128, D], BF16, tag="rsin", name="rsin")
        rs_out = dram.tile([4, 128, D], BF16, tag="rsout", name="rsout")

        nc.vector.memset(eps_s[:], 1e-5)
        nc.vector.memset(lnb_s[:], LNBIAS)
        make_identity(nc, ident[:])
        for h in range(HL):
            nc.vector.memset(v8[:, :, :, h, DH:DH + 1], 1.0 / 64.0)

        # ---------- long-lived working pools (stack: under ctp/attn-phase) ----
        psM_cm = tc.tile_pool(name="psM", bufs=2, space="PSUM")
        psM = psM_cm.__enter__()
        psT_cm = tc.tile_pool(name="psT", bufs=1, space="PSUM")
        psT = psT_cm.__enter__()
        w1p_cm = tc.tile_pool(name="w1p", bufs=4)
        w1p = w1p_cm.__enter__()
        w2p_cm = tc.tile_pool(name="w2p", bufs=8)
        w2p = w2p_cm.__enter__()
        lnw_cm = tc.tile_pool(name="lnw", bufs=1)
        lnw = lnw_cm.__enter__()

        # ---------- stage A: DMAs + projections ----------
        ctp_cm = tc.tile_pool(name="ctp", bufs=1)
        ctp = ctp_cm.__enter__()
        ct8 = [ctp.tile([128, 2, TM], FP8, tag=f"ct8_{p}", name=f"ct8_{p}")
               for p in range(4)]
        wq8 = [ctp.tile([128, 2, HDH_L], FP8, tag=f"wq{p}", name=f"wq{p}")
               for p in range(4)]
        wk8 = [ctp.tile([128, 2, HDH_L], FP8, tag=f"wk{p}", name=f"wk{p}")
               for p in range(4)]
        wv8 = [ctp.tile([128, 2, HDH_L], FP8, tag=f"wv{p}", name=f"wv{p}")
               for p in range(4)]
        for p in range(4):
            nc.sync.dma_start(out=ct8[p][:], in_=ct8_d[p])
            nc.sync.dma_start(out=wq8[p][:], in_=wq8_d[p])
            nc.sync.dma_start(out=wk8[p][:], in_=wk8_d[p])
            nc.sync.dma_start(out=wv8[p][:], in_=wv8_d[p])
        nc.sync.dma_start(out=wo8_s[:], in_=wo8_d[:])
        for i in range(len(mask_list)):
            nc.sync.dma_start(out=mk_s[i][:], in_=mask_d[i])
        nc.sync.dma_start(out=b1t_s[:], in_=b1t_d[:])
        nc.sync.dma_start(out=b2b_s[:], in_=b2b_d[:])
        for i in range(4):
            nc.sync.dma_start(out=ln_s[i][:], in_=lnp_d[i])
            nc.sync.dma_start(out=hres_s[i][:], in_=hres_d[i])

        # Q^T: [128 feat, T]; feature row f = 64h + d, m-tile = f//128
        for m in range(2):
            for n in range(4):
                pq = psM.tile([128, 512], F32, tag="pm", name="pm")
                for p in range(4):
                    nc.tensor.matmul(
                        pq[:], wq8[p][:, :, m * 128:(m + 1) * 128],
                        ct8[p][:, :, M + n * 512: M + (n + 1) * 512],
                        start=(p == 0), stop=(p == 3), perf_mode=DR,
                    )
                nc.vector.tensor_copy(qT[m][:, n * 512:(n + 1) * 512], pq[:])
        for m in range(2):
            for n in range(6):
                pk = psM.tile([128, 512], F32, tag="pm", name="pm")
                for p in range(4):
                    nc.tensor.matmul(
                        pk[:], wk8[p][:, :, m * 128:(m + 1) * 128],
                        ct8[p][:, :, n * 512:(n + 1) * 512],
                        start=(p == 0), stop=(p == 3), perf_mode=DR,
                    )
                nc.vector.tensor_copy(kT[m][:, n * 512:(n + 1) * 512], pk[:])
        # V natural: [j, (h,d)] via second projection; evict per head on ACT
        for jb in range(NB_J):
            pv = psM.tile([128, HDH_L], F32, tag="pm", name="pm")
            for p in range(4):
                nc.tensor.matmul(
                    pv[:], ct8[p][:, :, jb * 128:(jb + 1) * 128], wv8[p][:],
                    start=(p == 0), stop=(p == 3), perf_mode=DR,
                )
            for h in range(HL):
                nc.scalar.activation(
                    v8[:, jb // 2, jb % 2, h, 0:DH], pv[:, h * 64:(h + 1) * 64],
                    AF.Copy, bias=0.0, scale=1.0,
                )

        # ---------- deferred FF work (drained into attention PE gaps) ----------
        def layernorm(x_out, x_in, g_sb, b_sb):
            st = lnw.tile([128, 2, 6], F32, tag="bnst", name="bnst")
            for s in range(2):
                nc.vector.bn_stats(out=st[:, s, :],
                                   in_=x_in[:, s * 512:(s + 1) * 512])
            mv = lnw.tile([128, 2], F32, tag="bnmv", name="bnmv")
            nc.vector.bn_aggr(out=mv[:], in_=st[:])
            nc.scalar.activation(
                out=mv[:, 1:2], in_=mv[:, 1:2], func=AF.Sqrt,
                bias=eps_s[:, 0:1], scale=1.0,
            )
            nc.vector.reciprocal(out=mv[:, 1:2], in_=mv[:, 1:2])
            nc.vector.tensor_scalar(
                out=x_out, in0=x_in, scalar1=mv[:, 0:1], scalar2=mv[:, 1:2],
                op0=ALU.subtract, op1=ALU.mult,
            )
            nc.vector.tensor_mul(x_out, x_out, g_sb)
            nc.vector.tensor_add(x_out, x_out, b_sb)

        def q_ln(c):
            def go():
                asum = lnw.tile([128, D], BF16, tag="asum", name="asum")
                nc.sync.dma_start(out=asum[:], in_=rs_out[c])
                xin = lnw.tile([128, D], F32, tag="xin", name="xin")
                nc.vector.tensor_add(xin[:], asum[:], hres_s[c][:])
                layernorm(x_s[c][:], xin[:], ln_s[0][:], ln_s[1][:])
                for kd in range(8):
                    ptr = psT.tile([128, 128], BF16, tag="ptr", name="ptr")
                    nc.tensor.transpose(
                        ptr[:], x_s[c][:, kd * 128:(kd + 1) * 128], ident[:]
                    )
                    nc.vector.tensor_copy(xT[kd][:, c * 128:(c + 1) * 128], ptr[:])
            return go

        def q_ff1(dics, c0, w):
            # FF1 for dic list over xT cols [c0*128, (c0+w)*128)
            def go():
                lo, hi = c0 * 128, (c0 + w) * 128
                for dic in dics:
                    w1t = w1p.tile([128, 8, 128], BF16, tag="w1t", name="w1t")
                    nc.sync.dma_start(out=w1t[:], in_=w1_d[dic])
                    f1 = psM.tile([128, w * 128], F32, tag="pm", name="pm")
                    for kd in range(8):
                        nc.tensor.matmul(
                            f1[:], w1t[:, kd, :], xT[kd][:, lo:hi],
                            start=(kd == 0), stop=(kd == 7),
                        )
                    nc.vector.tensor_scalar(
                        out=rT[dic][:, lo:hi], in0=f1[:],
                        scalar1=b1t_s[:, dic:dic + 1], scalar2=0.0,
                        op0=ALU.add, op1=ALU.max,
                    )
            return go

        def q_ff2(grp, cbs, w2t_box):
            # FF2 partial for dic octet `grp` over chunks in cbs
            def go():
                if w2t_box[0] is None:
                    w2t_box[0] = []
                    for i in range(8):
                        w2t = w2p.tile([128, D], BF16, tag="w2t", name="w2t")
                        nc.sync.dma_start(out=w2t[:], in_=w2_d[grp * 8 + i])
                        w2t_box[0].append(w2t)
                for cb in cbs:
                    for nn in range(2):
                        f2p = psM.tile([128, 512], F32, tag="pm", name="pm")
                        for i in range(8):
                            dic = grp * 8 + i
                            nc.tensor.matmul(
                                f2p[:], rT[dic][:, cb * 128:(cb + 1) * 128],
                                w2t_box[0][i][:, nn * 512:(nn + 1) * 512],
                                start=(i == 0), stop=(i == 7),
                            )
                        dst = f2sb[cb][:, nn * 512:(nn + 1) * 512]
                        if grp == 0:
                            nc.vector.tensor_copy(dst, f2p[:])
                        else:
                            nc.vector.scalar_tensor_tensor(
                                out=dst, in0=f2p[:], scalar=1.0, in1=dst,
                                op0=ALU.mult, op1=ALU.add,
                            )
            return go

        def q_epi(c):
            def go():
                x2 = lnw.tile([128, D], F32, tag="x2", name="x2")
                nc.vector.tensor_add(x2[:], f2sb[c][:], x_s[c][:])
                nc.vector.tensor_add(x2[:], x2[:], b2b_s[:])
                o = lnw.tile([128, D], F32, tag="o", name="o")
                layernorm(o[:], x2[:], ln_s[2][:], ln_s[3][:])
                nc.sync.dma_start(out=out_d[c], in_=o[:])
            return go

        # queue: (ready_after_rs_chunk, closure)
        queue = []
        queue.append((0, q_ln(0)))
        queue.append((1, q_ln(1)))
        for g_ in range(4):
            queue.append((1, q_ff1(range(g_ * 8, g_ * 8 + 8), 0, 2)))
        for g_ in range(4):
            box = [None]
            queue.append((1, q_ff2(g_, [0], box)))
            queue.append((1, q_ff2(g_, [1], box)))
        queue.append((1, q_epi(0)))
        queue.append((1, q_epi(1)))
        queue.append((2, q_ln(2)))
        queue.append((3, q_ln(3)))
        for g_ in range(4):
            queue.append((3, q_ff1(range(g_ * 8, g_ * 8 + 8), 2, 2)))
        for g_ in range(4):
            box = [None]
            queue.append((3, q_ff2(g_, [2], box)))
            queue.append((3, q_ff2(g_, [3], box)))
        queue.append((3, q_epi(2)))
        queue.append((3, q_epi(3)))

        rs_done = [-1]  # last RS chunk EMITTED

        def pump(budget, cur_c, cur_h):
            # drain up to `budget` quanta whose RS dep was emitted at least
            # one head ago (so the collective has had time to land)
            n = 0
            while queue and n < budget:
                r, fn = queue[0]
                if r > rs_done[0]:
                    break
                if r == rs_done[0] and cur_c == r + 1 and cur_h < 1:
                    break
                queue.pop(0)
                fn()
                n += 1

        # ---------- attention + Wo + RS, chunk loop ----------
        ctp_cm.__exit__(None, None, None)
        psS_cm = tc.tile_pool(name="psS", bufs=2, space="PSUM")
        psS = psS_cm.__enter__()
        psA_cm = tc.tile_pool(name="psA", bufs=1, space="PSUM")
        psA = psA_cm.__enter__()
        ptp_cm = tc.tile_pool(name="ptp", bufs=2)
        ptp = ptp_cm.__enter__()
        avp_cm = tc.tile_pool(name="avp", bufs=2)
        avp = avp_cm.__enter__()
        nrm_cm = tc.tile_pool(name="nrm", bufs=2)
        nrm = nrm_cm.__enter__()
        aop_cm = tc.tile_pool(name="aop", bufs=3)
        aop = aop_cm.__enter__()

        for c in range(NC_T):
            jbs = jbs_of(c)
            pairs = [tuple(jbs[i:i + 2]) for i in range(0, len(jbs), 2)]
            cmasks = mask_by.get(c, [])
            avT8 = avp.tile([128, 2, 512], FP8, tag="avT8", name="avT8")
            for h in range(HL):
                hp, ho = h // 2, (h % 2) * 64
                acc = psA.tile([65, 512], F32, tag="acc", name="acc")

                def emit_S(pr):
                    sp = psS.tile([128, 2, 512], F32, tag="sp", name="sp")
                    for i, jb in enumerate(pr):
                        nc.tensor.matmul(
                            sp[:, i, :],
                            kT[hp][ho:ho + 64, jb * 128:(jb + 1) * 128],
                            qT[hp][ho:ho + 64, c * 512:(c + 1) * 512],
                            start=True, stop=True,
                        )
                        for (bj, tb, mi) in cmasks:
                            if bj == jb:
                                o0 = tb * 128 - c * 512
                                nc.vector.tensor_add(
                                    sp[:, i, o0:o0 + 128],
                                    sp[:, i, o0:o0 + 128],
                                    mk_s[mi][:],
                                )
                    return sp

                def emit_exp_pv(sp, pr, first, last):
                    w = len(pr)
                    pt = ptp.tile([128, 2, 512], FP8, tag="pt", name="pt")
                    nc.scalar.activation(
                        pt[:, 0:w, :], sp[:, 0:w, :], AF.Exp,
                        bias=lnb_s[:, 0:1], scale=SCALE,
                    )
                    if w == 2:
                        nc.tensor.matmul(
                            acc[:], v8[:, pr[0] // 2, :, h, 0:DH + 1], pt[:],
                            start=first, stop=last, perf_mode=DR,
                        )
                    else:
                        nc.tensor.matmul(
                            acc[:], v8[:, pr[0] // 2, pr[0] % 2, h, 0:DH + 1],
                            pt[:, 0, :], start=first, stop=last,
                        )

                pend = None
                for i, pr in enumerate(pairs):
                    cur = emit_S(pr)
                    if pend is not None:
                        emit_exp_pv(pend[0], pend[1], pend[2], False)
                    pend = (cur, pr, i == 0)
                emit_exp_pv(pend[0], pend[1], pend[2], True)

                rec = nrm.tile([1, 512], F32, tag="rec", name="rec")
                nc.vector.reciprocal(rec[:], acc[64:65, :])
                bc = nrm.tile([64, 512], F32, tag="bc", name="bc")
                nc.gpsimd.partition_broadcast(bc[:], rec[:])
                nc.vector.tensor_mul(
                    avT8[ho:ho + 64, hp, :], acc[0:64, :], bc[:]
                )
                pump(2, c, h)

            for tb in range(4):
                ao = aop.tile([128, D], BF16, tag="ao", name="ao")
                for nn in range(2):
                    po = psM.tile([128, 512], F32, tag="pm", name="pm")
                    nc.tensor.matmul(
                        po[:], avT8[:, :, tb * 128:(tb + 1) * 128],
                        wo8_s[:, :, nn * 512:(nn + 1) * 512],
                        start=True, stop=True, perf_mode=DR,
                    )
                    nc.vector.tensor_scalar(
                        out=ao[:, nn * 512:(nn + 1) * 512], in0=po[:],
                        scalar1=1.0 / 1024.0, scalar2=None, op0=ALU.mult,
                    )
                nc.sync.dma_start(out=rs_in[4 * c + tb], in_=ao[:])
            nc.gpsimd.collective_compute(
                "ReduceScatter", mybir.AluOpType.add,
                replica_groups=[[0, 1, 2, 3], [4, 5, 6, 7]],
                ins=[rs_in[4 * c: 4 * (c + 1)].opt()],
                outs=[rs_out[c: c + 1].opt()],
            )
            rs_done[0] = c

        # close attention-phase pools, drain the tail
        for cm in (aop_cm, nrm_cm, avp_cm, ptp_cm, psA_cm, psS_cm):
            cm.__exit__(None, None, None)

        while queue:
            _, fn = queue.pop(0)
            fn()

        for cm in (lnw_cm, w2p_cm, w1p_cm, psT_cm, psM_cm, attn_cm):
            cm.__exit__(None, None, None)

    nc.compile()
    return nc


def _analyze_mask(attn_mask):
    """Derive block structure from the actual mask input."""
    mb = attn_mask  # [T, TM, B] bool
    any_vis = ~mb.all(axis=2)  # visible in at least one batch
    nb_t = T // 128
    fvt = []
    for bj in range(NB_J):
        col = any_vis[:, bj * 128:(bj + 1) * 128]
        vis_tb = [tb for tb in range(nb_t)
                  if col[tb * 128:(tb + 1) * 128, :].any()]
        fvt.append(vis_tb[0] if vis_tb else 16)
    m_any = attn_mask.any(axis=2)
    need = []
    for bj in range(NB_J):
        if fvt[bj] >= 16:
            continue
        for tb in range(fvt[bj], nb_t):
            if m_any[tb * 128:(tb + 1) * 128, bj * 128:(bj + 1) * 128].any():
                need.append((bj, tb))
    return tuple(fvt), tuple(need)


def _prep_inputs(dec_inp, attn_mask, mems, Wq, Wkv, Wo, ln1_g, ln1_b, W1, b1,
                 W2, b2, ln2_g, ln2_b, mask_list):
    c_full = np.concatenate([_f32(mems), _f32(dec_inp)], axis=0)  # [TM, B, D]
    w1_r = _bf(np.asarray(W1, np.float32).reshape(8, 128, 32, 128)
               .transpose(2, 1, 0, 3))
    w2_r = _bf(np.asarray(W2, np.float32).reshape(32, 128, D))
    b1t = _f32(np.asarray(b1).reshape(32, 128).T)
    b2b = _bf(np.broadcast_to(np.asarray(b2)[None, :], (128, D)))
    lnp = _bf(np.stack([np.broadcast_to(np.asarray(v, np.float32)[None, :],
                                        (128, D))
                        for v in (ln1_g, ln1_b, ln2_g, ln2_b)]))
    dec32 = _f32(dec_inp)

    def dr_pack(w):  # [1024, X] -> [4, 128, 2, X]
        return np.ascontiguousarray(
            w.reshape(4, 2, 128, -1).transpose(0, 2, 1, 3))

    in_maps = []
    for core in range(NCORES):
        b, g = core // G, core % G
        cT = c_full[:, b, :].T  # [1024, TM]
        ct8 = _f8(dr_pack(cT))
        wq_c = _f8(dr_pack(np.asarray(Wq, np.float32)
                           [:, g * HDH_L:(g + 1) * HDH_L]))
        wk_c = _f8(dr_pack(np.asarray(Wkv, np.float32)
                           [:, g * HDH_L:(g + 1) * HDH_L]))
        wv_c = _f8(dr_pack(np.asarray(Wkv, np.float32)
                           [:, H * DH + g * HDH_L: H * DH + (g + 1) * HDH_L]))
        wo_c = _f8((16.0 * np.asarray(Wo, np.float32)
                    [g * HDH_L:(g + 1) * HDH_L, :])
                   .reshape(2, 128, D).transpose(1, 0, 2))
        rows = np.concatenate(
            [np.arange(512 * q + 128 * g, 512 * q + 128 * g + 128)
             for q in range(4)])
        hres = _bf(dec32[rows, b, :]).reshape(4, 128, D)
        n_mask = max(len(mask_list), 1)
        mk = np.zeros((n_mask, 128, 128), np.float32)
        for i, (bj, tb) in enumerate(mask_list):
            blk = attn_mask[tb * 128:(tb + 1) * 128,
                            bj * 128:(bj + 1) * 128, b]
            mk[i] = np.where(blk.T, NEG, 0.0).astype(np.float32)
        in_maps.append({
            "ct8": ct8, "wq8": wq_c, "wk8": wk_c, "wv8": wv_c, "wo8": wo_c,
            "w1": w1_r, "w2": w2_r, "maskt": _bf(mk), "hres": hres,
            "lnp": lnp, "b1t": b1t, "b2b": b2b,
        })
    return in_maps


def kernel(dec_inp, attn_mask, mems, Wq, Wkv, Wo, ln1_g, ln1_b, W1, b1, W2, b2,
           ln2_g, ln2_b, _trace=False, _trace_kwargs=None):
    attn_mask = np.asarray(attn_mask).astype(bool)
    fvt, mask_list = _analyze_mask(attn_mask)
    key = (fvt, mask_list)
    if key not in _prog_cache:
        _prog_cache[key] = build_program(fvt, mask_list)
    nc = _prog_cache[key]

    in_maps = _prep_inputs(dec_inp, attn_mask, mems, Wq, Wkv, Wo, ln1_g, ln1_b,
                           W1, b1, W2, b2, ln2_g, ln2_b, mask_list)
    kw = {}
    if _trace:
        kw = dict(trace=True, **(_trace_kwargs or {}))
    res = run_bass_kernel_spmd(nc, in_maps, list(range(NCORES)), **kw)
    out = np.zeros((T, B, D), np.float32)
    for core in range(NCORES):
        b, g = core // G, core % G
        rows = np.concatenate(
            [np.arange(512 * q + 128 * g, 512 * q + 128 * g + 128)
             for q in range(4)])
        out[rows, b, :] = np.asarray(res.results[core]["out"]).reshape(TQ, D)
    if _trace:
        return out, res
    return out


# revision 18
# speedup vs baseline: 1.1052x; 1.1052x over previous
"""Trainium2 Bass kernel for nn_DecoderLayer (Transformer-XL style decoder layer).

Sharding (8 cores = 2 batch groups x 4-way tensor parallel):
  core c: b = c // 4, g = c % 4
  - Attention: head-parallel. Each core computes its 4 heads (of 16) for its
    batch: Q^T/K^T via column-parallel Wq/Wkv; scores S^T[j, t] on PE; exp on
    ACT; P^T against [V|1] accumulates attn_vec^T plus the softmax denominator
    in one PSUM group; row-parallel Wo gives a partial attn_out.
  - ReduceScatter over each 4-core group sums the Wo partials and scatters
    t-rows: core g receives rows [512g, 512g+512).
  - FF is sequence-parallel on the core's own 512 rows with full W1/W2.
  - Causal structure: score blocks with j > t+M are never computed; boundary
    blocks get an additive -1e9 mask built on the host from the actual
    attn_mask input (arbitrary masks fall back to more mask blocks).
All matmuls in bf16 with fp32 PSUM accumulation; softmax/LN in fp32.
"""

import sys

sys.path.insert(0, "/opt/trn_rl_repo")

from contextlib import ExitStack

import numpy as np
import ml_dtypes

import concourse.bass as bass
import concourse.bacc as bacc
import concourse.mybir as mybir
import concourse.tile as tile
from concourse.bass_utils import run_bass_kernel_spmd
from concourse.masks import make_identity

T, M, B, D, H, DH, DI = 2048, 1024, 2, 1024, 16, 64, 4096
TM = T + M
NCORES = 8
G = 4                # tensor-parallel group size
HL = H // G          # 4 local heads
HDH_L = HL * DH      # 256 local q/k/v features
TQ = T // G          # 512 t-rows per core after ReduceScatter
NB_J = TM // 128     # 24 key blocks
NEG = -1.0e9
SCALE = 1.0 / float(DH) ** 0.5

BF16 = mybir.dt.bfloat16
F32 = mybir.dt.float32
NPBF16 = ml_dtypes.bfloat16

_prog_cache = {}


def _bf(x):
    return np.ascontiguousarray(np.asarray(x, dtype=np.float32).astype(NPBF16))


def _f32(x):
    return np.ascontiguousarray(np.asarray(x, dtype=np.float32))


def build_program(fvt, mask_list, trace=False):
    """fvt[bj] = first visible t-block (0..16; 16 = column fully masked).
    mask_list = tuple of (bj, tb) pairs needing an additive mask tile."""
    fvt = list(fvt)
    n_mask = max(len(mask_list), 1)
    AF = mybir.ActivationFunctionType
    ALU = mybir.AluOpType

    nc = bacc.Bacc(None, target_bir_lowering=False, num_devices=NCORES)

    ct_d = nc.dram_tensor("ct", [8, 128, TM], BF16, kind="ExternalInput")
    hres_d = nc.dram_tensor("hres", [4, 128, D], F32, kind="ExternalInput")
    wq_d = nc.dram_tensor("wq", [8, 128, HDH_L], BF16, kind="ExternalInput")
    wkv_d = nc.dram_tensor("wkv", [8, 128, 2 * HDH_L], BF16, kind="ExternalInput")
    wo_d = nc.dram_tensor("wo", [2, 128, D], BF16, kind="ExternalInput")
    w1_d = nc.dram_tensor("w1", [32, 128, 8, 128], BF16, kind="ExternalInput")
    w2_d = nc.dram_tensor("w2", [32, 128, D], BF16, kind="ExternalInput")
    mask_d = nc.dram_tensor("maskt", [n_mask, 128, 128], F32, kind="ExternalInput")
    b1_d = nc.dram_tensor("b1t", [128, 32], F32, kind="ExternalInput")
    b2_d = nc.dram_tensor("b2b", [128, D], F32, kind="ExternalInput")
    ln_d = nc.dram_tensor("lnp", [4, 128, D], F32, kind="ExternalInput")
    out_d = nc.dram_tensor("out", [4, 128, D], F32, kind="ExternalOutput")

    # last contributing bj per 512-wide accumulator piece (for stop= flags)
    last_bj = []
    for p in range(4):
        contrib = [bj for bj in range(NB_J) if fvt[bj] * 128 < (p + 1) * 512]
        last_bj.append(contrib[-1] if contrib else -1)

    mask_by_bj = {}
    for i, (bj, tb) in enumerate(mask_list):
        mask_by_bj.setdefault(bj, []).append((tb, i))

    with ExitStack() as ctx:
        tc = ctx.enter_context(tile.TileContext(nc))
        per = ctx.enter_context(tc.tile_pool(name="per", bufs=1))
        attn_cm = tc.tile_pool(name="attn", bufs=1)
        attn = attn_cm.__enter__()
        dram = ctx.enter_context(tc.tile_pool(name="dram", bufs=1, space="DRAM"))

        # ---- attention-lifetime SBUF tiles (pool closed after stage C)
        qT = [attn.tile([128, T], BF16, tag=f"qT{m}", name=f"qT{m}") for m in range(2)]
        kvT = [attn.tile([128, TM], BF16, tag=f"kvT{m}", name=f"kvT{m}") for m in range(4)]
        v_s = [attn.tile([128, HL, DH + 1], BF16, tag=f"v{jb}", name=f"v{jb}") for jb in range(NB_J)]
        avT = [attn.tile([128, T], BF16, tag=f"avT{m}", name=f"avT{m}") for m in range(2)]
        wo_s = [attn.tile([128, D], BF16, tag=f"wo{m}", name=f"wo{m}") for m in range(2)]
        mk_s = [attn.tile([128, 128], F32, tag=f"mk{i}", name=f"mk{i}") for i in range(len(mask_list))]
        b1_s = per.tile([128, 32], F32, tag="b1", name="b1")
        b2_s = per.tile([128, D], F32, tag="b2", name="b2")
        ln_s = [per.tile([128, D], F32, tag=f"ln{i}", name=f"ln{i}") for i in range(4)]
        hres_s = [per.tile([128, D], F32, tag=f"hres{i}", name=f"hres{i}") for i in range(4)]
        ones_s = attn.tile([1, 64], BF16, tag="ones", name="ones")
        eps_s = per.tile([128, 1], F32, tag="eps", name="eps")
        z65_s = attn.tile([128, 65], BF16, tag="z65", name="z65")
        zrhs_s = attn.tile([128, 512], BF16, tag="zrhs", name="zrhs")
        rec_s = attn.tile([1, T], F32, tag="rec", name="rec")
        recb_s = attn.tile([1, T], BF16, tag="recb", name="recb")
        ident = per.tile([128, 128], BF16, tag="ident", name="ident")

        rs_in = dram.tile([16, 128, D], BF16, tag="rsin", name="rsin")
        rs_out = dram.tile([4, 128, D], BF16, tag="rsout", name="rsout")

        nc.vector.memset(ones_s[:], 1.0)
        nc.vector.memset(eps_s[:], 1e-5)
        nc.vector.memset(z65_s[:], 0.0)
        nc.vector.memset(zrhs_s[:], 0.0)
        make_identity(nc, ident[:])

        # ================= Stage A: projections =================
        with tc.tile_pool(name="ctp", bufs=1) as ctp, \
             tc.tile_pool(name="wp", bufs=1) as wp, \
             tc.tile_pool(name="psA", bufs=2, space="PSUM") as psA:
            ct_s = [ctp.tile([128, TM], BF16, tag=f"ct{kd}", name=f"ct{kd}") for kd in range(8)]
            wq_s = [wp.tile([128, HDH_L], BF16, tag=f"wq{kd}", name=f"wq{kd}") for kd in range(8)]
            wkv_s = [wp.tile([128, 2 * HDH_L], BF16, tag=f"wkv{kd}", name=f"wkv{kd}")
                     for kd in range(8)]
            for kd in range(8):
                nc.sync.dma_start(out=ct_s[kd][:], in_=ct_d[kd])
                nc.sync.dma_start(out=wq_s[kd][:], in_=wq_d[kd])
                nc.sync.dma_start(out=wkv_s[kd][:], in_=wkv_d[kd])

            # parameter DMAs emitted after the critical-path inputs
            for m in range(2):
                nc.sync.dma_start(out=wo_s[m][:], in_=wo_d[m])
            for i in range(len(mask_list)):
                nc.sync.dma_start(out=mk_s[i][:], in_=mask_d[i])
            nc.sync.dma_start(out=b1_s[:], in_=b1_d[:])
            nc.sync.dma_start(out=b2_s[:], in_=b2_d[:])
            for i in range(4):
                nc.sync.dma_start(out=ln_s[i][:], in_=ln_d[i])
                nc.sync.dma_start(out=hres_s[i][:], in_=hres_d[i])

            # qT[m][:, n*512:+512] = sum_kd wq[kd][:, m-cols].T @ hT-part
            for m in range(2):
                for n in range(4):
                    pq = psA.tile([128, 512], F32, tag="pa", name="pa")
                    for kd in range(8):
                        nc.tensor.matmul(
                            pq[:],
                            wq_s[kd][:, m * 128:(m + 1) * 128],
                            ct_s[kd][:, M + n * 512: M + (n + 1) * 512],
                            start=(kd == 0), stop=(kd == 7),
                        )
                    nc.vector.tensor_copy(qT[m][:, n * 512:(n + 1) * 512], pq[:])
            for m in range(4):
                for n in range(6):
                    pkv = psA.tile([128, 512], F32, tag="pa", name="pa")
                    for kd in range(8):
                        nc.tensor.matmul(
                            pkv[:],
                            wkv_s[kd][:, m * 128:(m + 1) * 128],
                            ct_s[kd][:, n * 512:(n + 1) * 512],
                            start=(kd == 0), stop=(kd == 7),
                        )
                    nc.vector.tensor_copy(kvT[m][:, n * 512:(n + 1) * 512], pkv[:])

            # V natural layout via PE transpose of kvT rows 256..511
            for jb in range(NB_J):
                nc.vector.memset(v_s[jb][:, :, DH:DH + 1], 1.0)
                for vb in range(2):
                    ptr = psA.tile([128, 128], BF16, tag="ptr", name="ptr")
                    nc.tensor.transpose(
                        ptr[:], kvT[2 + vb][:, jb * 128:(jb + 1) * 128], ident[:]
                    )
                    for c_ in range(2):
                        h_loc = 2 * vb + c_
                        nc.vector.tensor_copy(
                            v_s[jb][:, h_loc, 0:DH], ptr[:, c_ * 64:(c_ + 1) * 64]
                        )

        # ========= Stage B+C: attention, Wo, chunked ReduceScatter =========
        # Split over t-halves: Wo + RS for half 0 overlap attention of half 1.
        with tc.tile_pool(name="psBs", bufs=2, space="PSUM") as psBs, \
             tc.tile_pool(name="psBa", bufs=1, space="PSUM") as psBa, \
             tc.tile_pool(name="psC", bufs=2, space="PSUM") as psC, \
             tc.tile_pool(name="ptp", bufs=3) as ptp, \
             tc.tile_pool(name="bcp", bufs=2) as bcp, \
             tc.tile_pool(name="aop", bufs=3) as aop:
            for half in range(2):
                th0, th1 = half * 1024, (half + 1) * 1024
                # last contributing bj per absolute 512-piece in this half
                lastb = []
                for p in range(2):
                    pe_end = th0 + (p + 1) * 512
                    contrib = [bj for bj in range(NB_J)
                               if fvt[bj] < 16 and fvt[bj] * 128 < pe_end]
                    lastb.append(contrib[-1] if contrib else -1)
                for h in range(HL):
                    hp, ho = h // 2, (h % 2) * 64
                    acc = psBa.tile([65, 1024], F32, tag="acc", name="acc")
                    for p in range(2):
                        nc.tensor.matmul(
                            acc[:, p * 512:(p + 1) * 512], z65_s[:], zrhs_s[:],
                            start=True, stop=(lastb[p] < 0),
                        )

                    def emit_scores(bj):
                        qs, qe = max(fvt[bj] * 128, th0), th1
                        sp = psBs.tile([128, 1024], F32, tag="sp", name="sp")
                        ss = qs
                        while ss < qe:
                            se = min(qe, ss + 512)
                            nc.tensor.matmul(
                                sp[:, ss - qs: se - qs],
                                kvT[hp][ho:ho + 64, bj * 128:(bj + 1) * 128],
                                qT[hp][ho:ho + 64, ss:se],
                                start=True, stop=True,
                            )
                            ss = se
                        for tb, mi in mask_by_bj.get(bj, []):
                            c0 = tb * 128
                            if qs <= c0 < qe:
                                nc.vector.tensor_add(
                                    sp[:, c0 - qs: c0 - qs + 128],
                                    sp[:, c0 - qs: c0 - qs + 128],
                                    mk_s[mi][:],
                                )
                        return bj, qs, qe, sp

                    def emit_exp_pv(job):
                        bj, qs, qe, sp = job
                        pt = ptp.tile([128, 1024], BF16, tag="pt", name="pt")
                        nc.scalar.activation(
                            pt[:, 0: qe - qs], sp[:, 0: qe - qs], AF.Exp,
                            bias=0.0, scale=SCALE,
                        )
                        ss = qs
                        while ss < qe:
                            se = min(qe, (ss // 512 + 1) * 512)
                            p = (ss - th0) // 512
                            nc.tensor.matmul(
                                acc[:, ss - th0: se - th0],
                                v_s[bj][:, h, :],
                                pt[:, ss - qs: se - qs],
                                start=False, stop=(bj == lastb[p]),
                            )
                            ss = se

                    # software pipeline: emit S(bj+1) before exp/PV(bj) so the
                    # in-order PE stream never stalls on ACT's exp latency
                    pend = None
                    for bj in range(NB_J):
                        if fvt[bj] >= 16 or fvt[bj] * 128 >= th1:
                            continue
                        cur = emit_scores(bj)
                        if pend is not None:
                            emit_exp_pv(pend)
                        pend = cur
                    if pend is not None:
                        emit_exp_pv(pend)
                    # normalize: attn_vec^T * (1/denom)
                    nc.vector.reciprocal(rec_s[:, th0:th1], acc[64:65, :])
                    nc.vector.tensor_copy(recb_s[:, th0:th1], rec_s[:, th0:th1])
                    for p in range(2):
                        a0 = th0 + p * 512
                        bc_ps = psBs.tile([64, 512], F32, tag="sp", name="sp")
                        nc.tensor.matmul(
                            bc_ps[:], ones_s[:], recb_s[:, a0:a0 + 512],
                            start=True, stop=True,
                        )
                        bc = bcp.tile([64, 512], F32, tag="bc", name="bc")
                        nc.vector.tensor_copy(bc[:], bc_ps[:])
                        nc.vector.tensor_mul(
                            avT[hp][ho:ho + 64, a0:a0 + 512],
                            acc[0:64, p * 512:(p + 1) * 512],
                            bc[:],
                        )
                # Wo partials for this half + RS chunks
                for tcb in range(8 * half, 8 * half + 8):
                    ao = aop.tile([128, D], BF16, tag="ao", name="ao")
                    for nn in range(2):
                        po = psC.tile([128, 512], F32, tag="po", name="po")
                        for hp in range(2):
                            nc.tensor.matmul(
                                po[:],
                                avT[hp][:, tcb * 128:(tcb + 1) * 128],
                                wo_s[hp][:, nn * 512:(nn + 1) * 512],
                                start=(hp == 0), stop=(hp == 1),
                            )
                        nc.vector.tensor_copy(ao[:, nn * 512:(nn + 1) * 512], po[:])
                    nc.sync.dma_start(out=rs_in[tcb], in_=ao[:])
                    if tcb % 4 == 3:
                        q = tcb // 4
                        nc.gpsimd.collective_compute(
                            "ReduceScatter", mybir.AluOpType.add,
                            replica_groups=[[0, 1, 2, 3], [4, 5, 6, 7]],
                            ins=[rs_in[4 * q: 4 * (q + 1)].opt()],
                            outs=[rs_out[q: q + 1].opt()],
                        )

        attn_cm.__exit__(None, None, None)

        # ============ Stage D: LN1, FF, LN2, out ============
        def layernorm(x_out, x_in, g_sb, b_sb, sp_pool):
            st = sp_pool.tile([128, 2, 6], F32, tag="bnst", name="bnst")
            for s in range(2):
                nc.vector.bn_stats(out=st[:, s, :],
                                   in_=x_in[:, s * 512:(s + 1) * 512])
            mv = sp_pool.tile([128, 2], F32, tag="bnmv", name="bnmv")
            nc.vector.bn_aggr(out=mv[:], in_=st[:])
            nc.scalar.activation(
                out=mv[:, 1:2], in_=mv[:, 1:2], func=AF.Sqrt,
                bias=eps_s[:, 0:1], scale=1.0,
            )
            nc.vector.reciprocal(out=mv[:, 1:2], in_=mv[:, 1:2])
            nc.vector.tensor_scalar(
                out=x_out, in0=x_in, scalar1=mv[:, 0:1], scalar2=mv[:, 1:2],
                op0=ALU.subtract, op1=ALU.mult,
            )
            nc.vector.tensor_mul(x_out, x_out, g_sb)
            nc.vector.tensor_add(x_out, x_out, b_sb)

        with tc.tile_pool(name="sdp", bufs=1) as sdp, \
             tc.tile_pool(name="sd", bufs=3) as sd:
            xT = [sdp.tile([128, TQ], BF16, tag=f"xT{k}", name=f"xT{k}") for k in range(8)]
            rT = [sdp.tile([128, TQ], BF16, tag=f"rT{k}", name=f"rT{k}") for k in range(32)]
            x_s = [sdp.tile([128, D], F32, tag=f"x{k}", name=f"x{k}") for k in range(4)]

            pstr_cm = tc.tile_pool(name="pstr", bufs=2, space="PSUM")
            pstr = pstr_cm.__enter__()
            for k4 in range(4):
                asum = sd.tile([128, D], BF16, tag="asum", name="asum")
                nc.sync.dma_start(out=asum[:], in_=rs_out[k4])
                xin = sd.tile([128, D], F32, tag="xin", name="xin")
                nc.vector.tensor_add(xin[:], asum[:], hres_s[k4][:])
                layernorm(x_s[k4][:], xin[:], ln_s[0][:], ln_s[1][:], sd)
                xbf = sd.tile([128, D], BF16, tag="xbf", name="xbf")
                nc.vector.tensor_copy(xbf[:], x_s[k4][:])
                for kd in range(8):
                    ptr = pstr.tile([128, 128], BF16, tag="tr", name="tr")
                    nc.tensor.transpose(
                        ptr[:], xbf[:, kd * 128:(kd + 1) * 128], ident[:]
                    )
                    nc.vector.tensor_copy(
                        xT[kd][:, k4 * 128:(k4 + 1) * 128], ptr[:]
                    )

            pstr_cm.__exit__(None, None, None)

            # FF1: rT[dic] = relu(W1[:, dic].T @ x^T + b1)
            with tc.tile_pool(name="w1p", bufs=3) as w1p, \
                 tc.tile_pool(name="psf1", bufs=3, space="PSUM") as psf1:
                for dic in range(32):
                    w1t = w1p.tile([128, 8, 128], BF16, tag="w1t", name="w1t")
                    nc.sync.dma_start(out=w1t[:], in_=w1_d[dic])
                    f1 = psf1.tile([128, TQ], F32, tag="f1", name="f1")
                    for kd in range(8):
                        nc.tensor.matmul(
                            f1[:], w1t[:, kd, :], xT[kd][:],
                            start=(kd == 0), stop=(kd == 7),
                        )
                    nc.scalar.activation(
                        out=rT[dic][:], in_=f1[:], func=AF.Relu,
                        bias=b1_s[:, dic:dic + 1], scale=1.0,
                    )

            # FF2 uses all 8 PSUM banks (4 accumulators of [128, 1024] f32)
            with tc.tile_pool(name="psF", bufs=1, space="PSUM") as psF, \
                 tc.tile_pool(name="w2p", bufs=3) as w2p, \
                 tc.tile_pool(name="se", bufs=2) as se:
                f2 = [psF.tile([128, D], F32, tag=f"f2_{k}", name=f"f2_{k}") for k in range(4)]
                for dic in range(32):
                    w2t = w2p.tile([128, D], BF16, tag="w2t", name="w2t")
                    nc.sync.dma_start(out=w2t[:], in_=w2_d[dic])
                    for k4 in range(4):
                        for nn in range(2):
                            nc.tensor.matmul(
                                f2[k4][:, nn * 512:(nn + 1) * 512],
                                rT[dic][:, k4 * 128:(k4 + 1) * 128],
                                w2t[:, nn * 512:(nn + 1) * 512],
                                start=(dic == 0), stop=(dic == 31),
                            )
                for k4 in range(4):
                    x2 = se.tile([128, D], F32, tag="x2", name="x2")
                    nc.vector.tensor_add(x2[:], f2[k4][:], x_s[k4][:])
                    nc.vector.tensor_add(x2[:], x2[:], b2_s[:])
                    o = se.tile([128, D], F32, tag="o", name="o")
                    layernorm(o[:], x2[:], ln_s[2][:], ln_s[3][:], se)
                    nc.sync.dma_start(out=out_d[k4], in_=o[:])

    nc.compile()
    return nc


def _analyze_mask(attn_mask):
    """Derive block structure from the actual mask input."""
    mb = attn_mask  # [T, TM, B] bool
    any_vis = ~mb.all(axis=2)  # visible in at least one batch
    nb_t = T // 128
    fvt = []
    for bj in range(NB_J):
        col = any_vis[:, bj * 128:(bj + 1) * 128]
        vis_tb = [tb for tb in range(nb_t)
                  if col[tb * 128:(tb + 1) * 128, :].any()]
        fvt.append(vis_tb[0] if vis_tb else 16)
    m_any = attn_mask.any(axis=2)
    need = []
    for bj in range(NB_J):
        if fvt[bj] >= 16:
            continue
        for tb in range(fvt[bj], nb_t):
            if m_any[tb * 128:(tb + 1) * 128, bj * 128:(bj + 1) * 128].any():
                need.append((bj, tb))
    return tuple(fvt), tuple(need)


def _prep_inputs(dec_inp, attn_mask, mems, Wq, Wkv, Wo, ln1_g, ln1_b, W1, b1,
                 W2, b2, ln2_g, ln2_b, mask_list):
    c_full = np.concatenate([_f32(mems), _f32(dec_inp)], axis=0)  # [TM, B, D]
    # w1_r[dic][p, kd, :] = W1[kd*128+p, dic*128:+128]
    w1_r = _bf(np.asarray(W1, np.float32).reshape(8, 128, 32, 128)
               .transpose(2, 1, 0, 3))
    w2_r = _bf(np.asarray(W2, np.float32).reshape(32, 128, D))
    b1t = _f32(np.asarray(b1).reshape(32, 128).T)
    b2b = _f32(np.broadcast_to(np.asarray(b2)[None, :], (128, D)))
    lnp = _f32(np.stack([np.broadcast_to(np.asarray(v, np.float32)[None, :],
                                         (128, D))
                         for v in (ln1_g, ln1_b, ln2_g, ln2_b)]))
    dec32 = _f32(dec_inp)
    in_maps = []
    for core in range(NCORES):
        b, g = core // G, core % G
        ctb = _bf(c_full[:, b, :].T.reshape(8, 128, TM))
        rows = np.concatenate(
            [np.arange(512 * q + 128 * g, 512 * q + 128 * g + 128)
             for q in range(4)])
        hres = _f32(dec32[rows, b, :]).reshape(4, 128, D)
        wq_c = _bf(np.asarray(Wq)[:, g * HDH_L:(g + 1) * HDH_L]
                   .reshape(8, 128, HDH_L))
        wkv_c = _bf(np.concatenate(
            [np.asarray(Wkv)[:, g * HDH_L:(g + 1) * HDH_L],
             np.asarray(Wkv)[:, H * DH + g * HDH_L: H * DH + (g + 1) * HDH_L]],
            axis=1).reshape(8, 128, 2 * HDH_L))
        wo_c = _bf(np.asarray(Wo)[g * HDH_L:(g + 1) * HDH_L, :]
                   .reshape(2, 128, D))
        n_mask = max(len(mask_list), 1)
        mk = np.zeros((n_mask, 128, 128), np.float32)
        for i, (bj, tb) in enumerate(mask_list):
            blk = attn_mask[tb * 128:(tb + 1) * 128,
                            bj * 128:(bj + 1) * 128, b]
            mk[i] = np.where(blk.T, NEG, 0.0).astype(np.float32)
        in_maps.append({
            "ct": ctb, "hres": hres, "wq": wq_c, "wkv": wkv_c, "wo": wo_c,
            "w1": w1_r, "w2": w2_r, "maskt": mk, "b1t": b1t, "b2b": b2b,
            "lnp": lnp,
        })
    return in_maps


def kernel(dec_inp, attn_mask, mems, Wq, Wkv, Wo, ln1_g, ln1_b, W1, b1, W2, b2,
           ln2_g, ln2_b, _trace=False, _trace_kwargs=None):
    attn_mask = np.asarray(attn_mask).astype(bool)
    fvt, mask_list = _analyze_mask(attn_mask)
    key = (fvt, mask_list)
    if key not in _prog_cache:
        _prog_cache[key] = build_program(fvt, mask_list)
    nc = _prog_cache[key]

    in_maps = _prep_inputs(dec_inp, attn_mask, mems, Wq, Wkv, Wo, ln1_g, ln1_b,
                           W1, b1, W2, b2, ln2_g, ln2_b, mask_list)
    kw = {}
    if _trace:
        kw = dict(trace=True, **(_trace_kwargs or {}))
    res = run_bass_kernel_spmd(nc, in_maps, list(range(NCORES)), **kw)
    out = np.zeros((T, B, D), np.float32)
    for core in range(NCORES):
        b, g = core // G, core % G
        rows = np.concatenate(
            [np.arange(512 * q + 128 * g, 512 * q + 128 * g + 128)
             for q in range(4)])
        out[rows, b, :] = np.asarray(res.results[core]["out"]).reshape(TQ, D)
    if _trace:
        return out, res
    return out

